# revision 11
# baseline (speedup 1.0000x reference)
"""Trainium2 Bass kernel for nn_AdditiveAttention (Bahdanau additive attention).

Distribution: head-parallel across 8 NeuronCores (H=8, one head per core).
Each core computes its head's additive-attention output heads_h^T [64, B*T],
chunked AllGathers concatenate heads over cores (row axis = h-major units)
overlapped with the main loop, and every core redundantly applies the output
projection; the host takes core 0's output.

Per-core dataflow (head h), B=2, T=512, D=512, DEPTH=64:
  1. Stream KEY tiles: DMA [128, 512] -> PE-transpose -> per-tile projection
     K_hT = Wk_s.T @ kT + bk (accumulated over D-chunks), then
     k2 [128, T] bf16 = (b0; b1)-packed Wk_h.T @ K_hT + b_h via col-tiled
     matmuls (partitions 0:64 = batch0, 64:128 = batch1).
  2. Stream QUERY tiles in (b0, b1) pairs producing qb2 [128, T] f32 chunks;
     the main loop starts as soon as the first chunk is ready.
  3. Slab stage over t (ACT-bound, the dominant cost):
       sum_slab[:, jT:(j+1)T] = k2 + qb2[:, t]     (DVE tensor_scalar, bf16)
       tanh_slab = tanh(sum_slab)                  (ACT, 1 elem/cycle/lane)
       score_ps += G_j.T @ tanh_slab_j             (PE, banded stationary)
     G [128, 254] holds va packed so slice G[:, 126-2j : 254-2j] has va at
     columns 2j (rows 0:64) and 2j+1 (rows 64:128): matmul j accumulates
     t's scores into PSUM rows 2j, 2j+1 and zeros elsewhere.
  4. Softmax over s (rows r=2j+bb are (t,b) pairs; exp can't overflow:
     |score| <= sum|va| ~ 2.6), attn -> bf16, PE-transpose into
     attnT [128, n_sp, n_g, 128] keeping the interleaved column order
     (contiguous drains); emission deferred into the next score tile so
     ACT never waits on PE.
  5. Every 2 score tiles (128 t's), a token-chunk pipeline overlapped with
     the main loop: heads^T chunk (PE, stride-2 column APs split batches)
     -> AllGather (TOPSP/SDMA, free) -> out chunk = mergedT.T @ Wo + bo ->
     DMA out. Only the last chunk's tail is exposed.
"""

import numpy as np

import concourse.bass as bass
import concourse.mybir as mybir
import concourse.tile as tile
from concourse import bacc
from concourse.bass_utils import run_bass_kernel_spmd
from concourse.masks import make_identity

FP32 = mybir.dt.float32
BF16 = mybir.dt.bfloat16

NCORES = 8
B = 2
D = 512
UNITS = 512
H = 8
DEPTH = 64
GT = 16  # t-columns per tanh slab group

Tanh = mybir.ActivationFunctionType.Tanh
Exp = mybir.ActivationFunctionType.Exp
Identity = mybir.ActivationFunctionType.Identity


def build_nc(T=512):
    tokens = B * T
    n_sp = T // 128        # s-partition chunks
    n_g = T // 64          # score tiles (64 t's each)
    n_m = tokens // 128    # token tiles
    n_ch = T // 128        # token-chunks for the heads/AG/out pipeline
    assert T % 128 == 0 and 64 % GT == 0

    nc = bacc.Bacc("TRN2", target_bir_lowering=False, debug=False,
                   num_devices=NCORES)

    q_d = nc.dram_tensor("query", [tokens, D], FP32, kind="ExternalInput")
    k_d = nc.dram_tensor("key", [tokens, D], FP32, kind="ExternalInput")
    wq_d = nc.dram_tensor("wq_s", [D, DEPTH], FP32, kind="ExternalInput")
    wk_d = nc.dram_tensor("wk_s", [D, DEPTH], FP32, kind="ExternalInput")
    bq_d = nc.dram_tensor("bq_s", [DEPTH, 1], FP32, kind="ExternalInput")
    bk_d = nc.dram_tensor("bk_s", [DEPTH, 1], FP32, kind="ExternalInput")
    wqh_d = nc.dram_tensor("wq_h", [DEPTH, DEPTH], FP32, kind="ExternalInput")
    wkh_d = nc.dram_tensor("wk_h", [DEPTH, DEPTH], FP32, kind="ExternalInput")
    va_d = nc.dram_tensor("va", [DEPTH, 1], FP32, kind="ExternalInput")
    bh_d = nc.dram_tensor("bh", [DEPTH, 1], FP32, kind="ExternalInput")
    wo_d = nc.dram_tensor("wo", [UNITS, UNITS], FP32, kind="ExternalInput")
    bo_d = nc.dram_tensor("bo", [1, UNITS], FP32, kind="ExternalInput")
    out_d = nc.dram_tensor("out", [tokens, UNITS], FP32, kind="ExternalOutput")

    with tile.TileContext(nc) as tc:
        with tc.tile_pool(name="consts", bufs=1) as consts, \
             tc.tile_pool(name="io", bufs=3) as io, \
             tc.tile_pool(name="slabs", bufs=2) as slabs, \
             tc.tile_pool(name="sm", bufs=2) as sm, \
             tc.tile_pool(name="outp", bufs=2) as outp, \
             tc.tile_pool(name="ps", bufs=2, space="PSUM") as ps, \
             tc.tile_pool(name="dram", bufs=1, space="DRAM") as dram:

            # ---------- small constants ----------
            id_f32 = consts.tile([128, 128], FP32)
            make_identity(nc, id_f32)
            id_bf16 = consts.tile([128, 128], BF16)
            make_identity(nc, id_bf16)

            # banded va matrix G: G[0:64, 126] = va, G[64:128, 127] = va
            va_g = consts.tile([128, 254], BF16)
            nc.vector.memset(va_g, 0.0)
            vtmp2 = consts.tile([128, 2], FP32)
            nc.vector.memset(vtmp2, 0.0)
            nc.gpsimd.dma_start(out=vtmp2[0:64, 0:1], in_=va_d[:, :])
            nc.gpsimd.dma_start(out=vtmp2[64:128, 1:2], in_=va_d[:, :])
            nc.vector.tensor_copy(va_g[:, 126:128], vtmp2)

            # b_h stacked twice (per-partition bias for k2)
            b2col = consts.tile([128, 1], FP32)
            nc.gpsimd.dma_start(out=b2col[0:64, :], in_=bh_d[:, :])
            nc.gpsimd.dma_start(out=b2col[64:128, :], in_=bh_d[:, :])

            # projection weights (small)
            wq_sb = consts.tile([128, 4, DEPTH], FP32)
            nc.gpsimd.dma_start(out=wq_sb, in_=wq_d.rearrange("(k p) j -> p k j", p=128))
            wk_sb = consts.tile([128, 4, DEPTH], FP32)
            nc.gpsimd.dma_start(out=wk_sb, in_=wk_d.rearrange("(k p) j -> p k j", p=128))
            wqh_sb = consts.tile([DEPTH, DEPTH], FP32)
            nc.gpsimd.dma_start(out=wqh_sb, in_=wqh_d[:, :])
            wkh_sb = consts.tile([DEPTH, DEPTH], FP32)
            nc.gpsimd.dma_start(out=wkh_sb, in_=wkh_d[:, :])
            bq_sb = consts.tile([DEPTH, 1], FP32)
            nc.gpsimd.dma_start(out=bq_sb, in_=bq_d[:, :])
            bk_sb = consts.tile([DEPTH, 1], FP32)
            nc.gpsimd.dma_start(out=bk_sb, in_=bk_d[:, :])

            # persistent intermediates
            KhT = consts.tile([DEPTH, tokens], FP32)
            QhT = consts.tile([DEPTH, tokens], FP32)
            qb2 = consts.tile([128, T], FP32)
            k2 = consts.tile([128, T], BF16)
            khb = consts.tile([128, B, n_sp, DEPTH], BF16)
            attnT = consts.tile([128, n_sp, n_g, 128], BF16)
            headsT = consts.tile([DEPTH, B, T], BF16)

            def project_tile(qk_tile, m, w_sb, b_sb, dsth, on_act):
                """Transpose a loaded token tile + project to dsth[:, 128m:...].

                on_act: route drains through ScalarE (idle pre-loop) or DVE
                (mid-loop, so ACT's strict FIFO never waits on this chain)."""
                tp = ps.tile([128, 512], FP32, tag="tp", bufs=2, name="tp")
                for k in range(4):
                    nc.tensor.transpose(tp[:, 128 * k:128 * (k + 1)],
                                        qk_tile[:, 128 * k:128 * (k + 1)], id_f32)
                tT = io.tile([128, 4, 128], FP32, tag="tT", name="tT")
                tp_r = tp.rearrange("p (k i) -> p k i", k=4)
                if on_act:
                    nc.scalar.copy(tT, tp_r)
                else:
                    nc.vector.tensor_copy(tT, tp_r)
                pj = ps.tile([DEPTH, 128], FP32, tag="pj", bufs=2, name="pj")
                for k in range(4):
                    nc.tensor.matmul(pj, lhsT=w_sb[:, k, :], rhs=tT[:, k, :],
                                     start=(k == 0), stop=(k == 3))
                dst = dsth[:, 128 * m:128 * (m + 1)]
                if on_act:
                    nc.scalar.activation(dst, pj, Identity, bias=b_sb)
                else:
                    nc.vector.tensor_scalar_add(dst, pj, b_sb)

            # ONE big DMA each for key (sync/HWDGE) and query (gpsimd/SWDGE):
            # a single dma_start fans its descriptors across the HW queues,
            # while many small dma_starts serialize on the ~0.6us-per-dispatch
            # sequencer. The two engines stream in parallel.
            kbig = consts.tile([128, n_m, D], FP32)
            nc.sync.dma_start(out=kbig,
                              in_=k_d.rearrange("(m p) d -> p m d", p=128))
            qbig = consts.tile([128, n_m, D], FP32)
            nc.gpsimd.dma_start(out=qbig,
                                in_=q_d.rearrange("(m p) d -> p m d", p=128))

            # ---------- key path (complete before the main loop) ----------
            for m in range(n_m):
                project_tile(kbig[:, m, :], m, wk_sb, bk_sb, KhT, on_act=True)
            psk2 = ps.tile([128, T], FP32, tag="big", bufs=2, name="psk2")
            nc.tensor.matmul(psk2[0:64, :], lhsT=wkh_sb, rhs=KhT[:, 0:T],
                             start=True, stop=True)
            nc.tensor.matmul(psk2[64:128, :], lhsT=wkh_sb, rhs=KhT[:, T:2 * T],
                             start=True, stop=True)
            nc.scalar.activation(k2, psk2, Identity, bias=b2col)
            # K_h token-major (lhsT of the heads matmul), bf16
            for bb in range(B):
                for k in range(n_sp):
                    tp2 = ps.tile([128, 512], FP32, tag="tp", bufs=2, name="tp2")
                    nc.tensor.transpose(
                        tp2[:, 0:DEPTH],
                        KhT[:, bb * T + 128 * k: bb * T + 128 * (k + 1)],
                        id_f32[0:64, 0:64])
                    nc.vector.tensor_copy(khb[:, bb, k, :], tp2[:, 0:DEPTH])

            def emit_query_pair(c, on_act):
                for mm in (c, n_ch + c):
                    project_tile(qbig[:, mm, :], mm, wq_sb, bq_sb, QhT, on_act)
                psqb = ps.tile([128, 128], FP32, tag="pj", bufs=2, name="psqb")
                nc.tensor.matmul(psqb[0:64, :], lhsT=wqh_sb,
                                 rhs=QhT[:, 128 * c:128 * (c + 1)],
                                 start=True, stop=True)
                nc.tensor.matmul(psqb[64:128, :], lhsT=wqh_sb,
                                 rhs=QhT[:, T + 128 * c:T + 128 * (c + 1)],
                                 start=True, stop=True)
                dst = qb2[:, 128 * c:128 * (c + 1)]
                if on_act:
                    nc.scalar.copy(dst, psqb)
                else:
                    nc.vector.tensor_copy(dst, psqb)

            emit_query_pair(0, on_act=True)

            # output-projection constants, emitted AFTER the key stream so
            # the 1MB Wo load queues behind it (needed only at ~100us)
            wo_sb = consts.tile([128, 4, UNITS], FP32)
            nc.sync.dma_start(out=wo_sb, in_=wo_d.rearrange("(k p) n -> p k n", p=128))
            wo_bf = consts.tile([128, 4, UNITS], BF16)
            nc.vector.tensor_copy(wo_bf, wo_sb)
            bo_bc = consts.tile([128, UNITS], FP32)
            bo_bcast_ap = bass.AP(tensor=bo_d.ap().tensor, offset=0,
                                  ap=[[0, 128], [1, UNITS]])
            nc.sync.dma_start(out=bo_bc, in_=bo_bcast_ap)

            # ---------- main loop with streamed query + deferred stages ----
            def make_softmax(g):
                def emit():
                    score_tile = score_tiles.pop(g)
                    probs = sm.tile([128, T], FP32, tag="probs", name="probs")
                    nc.scalar.activation(probs, score_tile, Exp)
                    sums = sm.tile([128, 1], FP32, tag="sums", name="sums")
                    nc.vector.reduce_sum(sums, probs, axis=mybir.AxisListType.X)
                    rsum = sm.tile([128, 1], FP32, tag="rsum", name="rsum")
                    nc.vector.reciprocal(rsum, sums)
                    attn = sm.tile([128, T], BF16, tag="attn", name="attn")
                    nc.vector.tensor_scalar_mul(attn, probs, rsum)
                    tpsb = ps.tile([128, T], BF16, tag="tp", bufs=2, name="tpsb")
                    for k in range(n_sp):
                        nc.tensor.transpose(tpsb[:, 128 * k:128 * (k + 1)],
                                            attn[:, 128 * k:128 * (k + 1)],
                                            id_bf16)
                    nc.vector.tensor_copy(
                        attnT[:, :, g, :],
                        tpsb.rearrange("p (k r) -> p k r", k=n_sp))
                return emit

            attnT_jb = attnT.rearrange("p k g (j b) -> p k g j b", b=B)

            def make_chunk(g):
                def emit():
                    t0c = 64 * g
                    for bb in range(B):
                        psh = ps.tile([DEPTH, 64], FP32, tag="pj", bufs=2,
                                      name="psh")
                        for k in range(n_sp):
                            nc.tensor.matmul(
                                psh, lhsT=khb[:, bb, k, :],
                                rhs=attnT_jb[:, k, g, :, bb],
                                start=(k == 0), stop=(k == n_sp - 1))
                        nc.vector.tensor_copy(headsT[:, bb, t0c:t0c + 64], psh)
                    hb = dram.tile([DEPTH, B, 64], BF16, tag="hb", bufs=4,
                                   name="hb")
                    ms = dram.tile([NCORES * DEPTH, B, 64], BF16,
                                   addr_space="Shared", tag="ms", bufs=4,
                                   name="ms")
                    nc.sync.dma_start(out=hb, in_=headsT[:, :, t0c:t0c + 64])
                    nc.gpsimd.collective_compute(
                        "AllGather", mybir.AluOpType.bypass,
                        replica_groups=[list(range(NCORES))],
                        ins=[hb.opt()], outs=[ms.opt()])
                    merged_c = io.tile([128, 4, B, 64], BF16, tag="merged_c",
                                       name="merged_c")
                    nc.sync.dma_start(
                        out=merged_c,
                        in_=ms.rearrange("(k p) b t -> p k b t", p=128))
                    for bb in range(B):
                        ops = ps.tile([DEPTH, UNITS], FP32, tag="big", bufs=2,
                                      name="ops")
                        for kc in range(4):
                            nc.tensor.matmul(ops, lhsT=merged_c[:, kc, bb, :],
                                             rhs=wo_bf[:, kc, :],
                                             start=(kc == 0), stop=(kc == 3))
                        out_sb = outp.tile([DEPTH, UNITS], FP32, tag="out_sb",
                                           name="out_sb")
                        nc.vector.tensor_add(out_sb, ops, bo_bc[0:DEPTH, :])
                        nc.sync.dma_start(
                            out=out_d[bb * T + t0c:bb * T + t0c + 64, :],
                            in_=out_sb)
                return emit

            score_tiles = {}
            pending = []  # deferred softmax/chunk emitters
            for g in range(n_g):
                if g % 2 == 1 and g // 2 + 1 < n_ch:
                    # stream the NEXT chunk's query pair a full score tile
                    # ahead of its first use (drains on DVE, off ACT's FIFO)
                    emit_query_pair(g // 2 + 1, on_act=False)

                score_ps = ps.tile([128, T], FP32, tag="score", bufs=2,
                                   name="score_ps")
                score_tiles[g] = score_ps
                for grp in range(64 // GT):
                    sum_slab = slabs.tile([128, GT * T], BF16, tag="sum_slab",
                                          name="sum_slab")
                    for j in range(GT):
                        t = 64 * g + GT * grp + j
                        nc.vector.tensor_scalar_add(
                            sum_slab[:, j * T:(j + 1) * T], k2, qb2[:, t:t + 1])
                    tanh_slab = slabs.tile([128, GT * T], BF16, tag="tanh_slab",
                                           name="tanh_slab", bufs=3)
                    nc.scalar.activation(tanh_slab, sum_slab, Tanh)
                    for j in range(GT):
                        jj = GT * grp + j  # t index within this score tile
                        nc.tensor.matmul(
                            score_ps,
                            lhsT=va_g[:, 126 - 2 * jj:254 - 2 * jj],
                            rhs=tanh_slab[:, j * T:(j + 1) * T],
                            start=(jj == 0), stop=(jj == 63))
                    if grp == 0:
                        for fn in pending:
                            fn()
                        pending = []
                pending.append(make_softmax(g))
                pending.append(make_chunk(g))
            for fn in pending:
                fn()

    nc.compile()
    return nc


def make_in_maps(inputs, T=512):
    """Shard full inputs head-parallel: core h gets head h's parameters."""
    f32 = np.float32
    q = np.ascontiguousarray(np.asarray(inputs["query"], f32)[:, :T, :].reshape(B * T, D))
    k = np.ascontiguousarray(np.asarray(inputs["key"], f32)[:, :T, :].reshape(B * T, D))
    Wq = np.asarray(inputs["Wq"], f32)
    Wk = np.asarray(inputs["Wk"], f32)
    bq = np.asarray(inputs["bq"], f32)
    bk = np.asarray(inputs["bk"], f32)
    Wq_h = np.asarray(inputs["Wq_h"], f32)
    Wk_h = np.asarray(inputs["Wk_h"], f32)
    va_h = np.asarray(inputs["va_h"], f32)
    b_h = np.asarray(inputs["b_h"], f32)
    Wo = np.ascontiguousarray(np.asarray(inputs["Wo"], f32))
    bo = np.ascontiguousarray(np.asarray(inputs["bo"], f32).reshape(1, UNITS))

    in_maps = []
    for h in range(NCORES):
        sl = slice(h * DEPTH, (h + 1) * DEPTH)
        in_maps.append({
            "query": q,
            "key": k,
            "wq_s": np.ascontiguousarray(Wq[:, sl]),
            "wk_s": np.ascontiguousarray(Wk[:, sl]),
            "bq_s": np.ascontiguousarray(bq[sl].reshape(DEPTH, 1)),
            "bk_s": np.ascontiguousarray(bk[sl].reshape(DEPTH, 1)),
            "wq_h": np.ascontiguousarray(Wq_h[h]),
            "wk_h": np.ascontiguousarray(Wk_h[h]),
            "va": np.ascontiguousarray(va_h[h].reshape(DEPTH, 1)),
            "bh": np.ascontiguousarray(b_h[h].reshape(DEPTH, 1)),
            "wo": Wo,
            "bo": bo,
        })
    return in_maps


_NC_CACHE = {}


def kernel(**inputs) -> np.ndarray:
    T = 512
    if T not in _NC_CACHE:
        _NC_CACHE[T] = build_nc(T)
    nc = _NC_CACHE[T]
    in_maps = make_in_maps(inputs, T)
    res = run_bass_kernel_spmd(nc, in_maps, core_ids=list(range(NCORES)))
    out = np.asarray(res.results[0]["out"], np.float32)
    return out.reshape(B, T, UNITS)


if __name__ == "__main__":
    import reference
    inp = {k: np.asarray(v) for k, v in reference.setup_inputs().items()}
    expected = np.asarray(reference.reference(**inp))
    got = kernel(**inp)
    rel = np.linalg.norm(got - expected) / np.linalg.norm(expected)
    print("Relative error:", rel)


# revision 13
# speedup vs baseline: 1.0230x; 1.0230x over previous
"""Trainium2 Bass kernel for nn_AdditiveAttention (Bahdanau additive attention).

Distribution: head-parallel across 8 NeuronCores (H=8, one head per core).
Each core computes its head's additive-attention output heads_h^T [64, B*T],
chunked AllGathers concatenate heads over cores (row axis = h-major units)
overlapped with the main loop, and every core redundantly applies the output
projection; the host takes core 0's output.

Per-core dataflow (head h), B=2, T=512, D=512, DEPTH=64:
  1. Stream KEY tiles: DMA [128, 512] -> PE-transpose -> per-tile projection
     K_hT = Wk_s.T @ kT + bk (accumulated over D-chunks), then
     k2 [128, T] bf16 = (b0; b1)-packed Wk_h.T @ K_hT + b_h via col-tiled
     matmuls (partitions 0:64 = batch0, 64:128 = batch1).
  2. Stream QUERY tiles in (b0, b1) pairs producing qb2 [128, T] f32 chunks;
     the main loop starts as soon as the first chunk is ready.
  3. Slab stage over t (ACT-bound, the dominant cost):
       sum_slab[:, jT:(j+1)T] = k2 + qb2[:, t]     (DVE tensor_scalar, bf16)
       tanh_slab = tanh(sum_slab)                  (ACT, 1 elem/cycle/lane)
       score_ps += G_j.T @ tanh_slab_j             (PE, banded stationary)
     G [128, 254] holds va packed so slice G[:, 126-2j : 254-2j] has va at
     columns 2j (rows 0:64) and 2j+1 (rows 64:128): matmul j accumulates
     t's scores into PSUM rows 2j, 2j+1 and zeros elsewhere.
  4. Softmax over s (rows r=2j+bb are (t,b) pairs; exp can't overflow:
     |score| <= sum|va| ~ 2.6), attn -> bf16, PE-transpose into
     attnT [128, n_sp, n_g, 128] keeping the interleaved column order
     (contiguous drains); emission deferred into the next score tile so
     ACT never waits on PE.
  5. Every 2 score tiles (128 t's), a token-chunk pipeline overlapped with
     the main loop: heads^T chunk (PE, stride-2 column APs split batches)
     -> AllGather (TOPSP/SDMA, free) -> out chunk = mergedT.T @ Wo + bo ->
     DMA out. Only the last chunk's tail is exposed.
"""

import numpy as np

import concourse.bass as bass
import concourse.mybir as mybir
import concourse.tile as tile
from concourse import bacc
from concourse.bass_utils import run_bass_kernel_spmd
from concourse.masks import make_identity

FP32 = mybir.dt.float32
BF16 = mybir.dt.bfloat16

NCORES = 8
B = 2
D = 512
UNITS = 512
H = 8
DEPTH = 64
GT = 16  # t-columns per tanh slab group

Tanh = mybir.ActivationFunctionType.Tanh
Exp = mybir.ActivationFunctionType.Exp
Identity = mybir.ActivationFunctionType.Identity


def build_nc(T=512):
    tokens = B * T
    n_sp = T // 128        # s-partition chunks
    n_g = T // 64          # score tiles (64 t's each)
    n_m = tokens // 128    # token tiles
    n_ch = T // 128        # token-chunks for the heads/AG/out pipeline
    assert T % 128 == 0 and 64 % GT == 0

    nc = bacc.Bacc("TRN2", target_bir_lowering=False, debug=False,
                   num_devices=NCORES)

    q_d = nc.dram_tensor("query", [tokens, D], FP32, kind="ExternalInput")
    k_d = nc.dram_tensor("key", [tokens, D], FP32, kind="ExternalInput")
    wq_d = nc.dram_tensor("wq_s", [D, DEPTH], FP32, kind="ExternalInput")
    wk_d = nc.dram_tensor("wk_s", [D, DEPTH], FP32, kind="ExternalInput")
    bq_d = nc.dram_tensor("bq_s", [DEPTH, 1], FP32, kind="ExternalInput")
    bk_d = nc.dram_tensor("bk_s", [DEPTH, 1], FP32, kind="ExternalInput")
    wqh_d = nc.dram_tensor("wq_h", [DEPTH, DEPTH], FP32, kind="ExternalInput")
    wkh_d = nc.dram_tensor("wk_h", [DEPTH, DEPTH], FP32, kind="ExternalInput")
    va_d = nc.dram_tensor("va", [DEPTH, 1], FP32, kind="ExternalInput")
    bh_d = nc.dram_tensor("bh", [DEPTH, 1], FP32, kind="ExternalInput")
    wo_d = nc.dram_tensor("wo", [UNITS, UNITS], FP32, kind="ExternalInput")
    bo_d = nc.dram_tensor("bo", [1, UNITS], FP32, kind="ExternalInput")
    out_d = nc.dram_tensor("out", [tokens, UNITS], FP32, kind="ExternalOutput")

    with tile.TileContext(nc) as tc:
        with tc.tile_pool(name="consts", bufs=1) as consts, \
             tc.tile_pool(name="io", bufs=3) as io, \
             tc.tile_pool(name="slabs", bufs=2) as slabs, \
             tc.tile_pool(name="sm", bufs=2) as sm, \
             tc.tile_pool(name="outp", bufs=2) as outp, \
             tc.tile_pool(name="ps", bufs=2, space="PSUM") as ps, \
             tc.tile_pool(name="dram", bufs=1, space="DRAM") as dram:

            # ---------- small constants ----------
            id_f32 = consts.tile([128, 128], FP32)
            make_identity(nc, id_f32)
            id_bf16 = consts.tile([128, 128], BF16)
            make_identity(nc, id_bf16)

            # banded va matrix G: G[0:64, 126] = va, G[64:128, 127] = va
            va_g = consts.tile([128, 254], BF16)
            nc.vector.memset(va_g, 0.0)
            vtmp2 = consts.tile([128, 2], FP32)
            nc.vector.memset(vtmp2, 0.0)
            nc.gpsimd.dma_start(out=vtmp2[0:64, 0:1], in_=va_d[:, :])
            nc.gpsimd.dma_start(out=vtmp2[64:128, 1:2], in_=va_d[:, :])
            nc.vector.tensor_copy(va_g[:, 126:128], vtmp2)

            # b_h stacked twice (per-partition bias for k2)
            b2col = consts.tile([128, 1], FP32)
            nc.gpsimd.dma_start(out=b2col[0:64, :], in_=bh_d[:, :])
            nc.gpsimd.dma_start(out=b2col[64:128, :], in_=bh_d[:, :])

            # projection weights (small)
            wq_sb = consts.tile([128, 4, DEPTH], FP32)
            nc.gpsimd.dma_start(out=wq_sb, in_=wq_d.rearrange("(k p) j -> p k j", p=128))
            wk_sb = consts.tile([128, 4, DEPTH], FP32)
            nc.gpsimd.dma_start(out=wk_sb, in_=wk_d.rearrange("(k p) j -> p k j", p=128))
            wqh_sb = consts.tile([DEPTH, DEPTH], FP32)
            nc.gpsimd.dma_start(out=wqh_sb, in_=wqh_d[:, :])
            wkh_sb = consts.tile([DEPTH, DEPTH], FP32)
            nc.gpsimd.dma_start(out=wkh_sb, in_=wkh_d[:, :])
            bq_sb = consts.tile([DEPTH, 1], FP32)
            nc.gpsimd.dma_start(out=bq_sb, in_=bq_d[:, :])
            bk_sb = consts.tile([DEPTH, 1], FP32)
            nc.gpsimd.dma_start(out=bk_sb, in_=bk_d[:, :])

            # persistent intermediates
            KhT = consts.tile([DEPTH, tokens], FP32)
            QhT = consts.tile([DEPTH, tokens], FP32)
            qb2 = consts.tile([128, T], FP32)
            k2 = consts.tile([128, T], BF16)
            khb = consts.tile([128, B, n_sp, DEPTH], BF16)
            attnT = consts.tile([128, n_sp, n_g, 128], BF16)
            headsT = consts.tile([DEPTH, B, T], BF16)

            def project_tile(qk_tile, m, w_sb, b_sb, dsth, on_act):
                """Transpose a loaded token tile + project to dsth[:, 128m:...].

                on_act: route drains through ScalarE (idle pre-loop) or DVE
                (mid-loop, so ACT's strict FIFO never waits on this chain)."""
                tp = ps.tile([128, 512], FP32, tag="tp", bufs=4, name="tp")
                for k in range(4):
                    nc.tensor.transpose(tp[:, 128 * k:128 * (k + 1)],
                                        qk_tile[:, 128 * k:128 * (k + 1)], id_f32)
                tT = io.tile([128, 4, 128], FP32, tag="tT", name="tT")
                tp_r = tp.rearrange("p (k i) -> p k i", k=4)
                if on_act:
                    nc.scalar.copy(tT, tp_r)
                else:
                    nc.vector.tensor_copy(tT, tp_r)
                pj = ps.tile([DEPTH, 128], FP32, tag="pj", bufs=2, name="pj")
                for k in range(4):
                    nc.tensor.matmul(pj, lhsT=w_sb[:, k, :], rhs=tT[:, k, :],
                                     start=(k == 0), stop=(k == 3))
                dst = dsth[:, 128 * m:128 * (m + 1)]
                if on_act:
                    nc.scalar.activation(dst, pj, Identity, bias=b_sb)
                else:
                    nc.vector.tensor_scalar_add(dst, pj, b_sb)

            # ONE big DMA each for key (sync/HWDGE) and query (gpsimd/SWDGE):
            # a single dma_start fans its descriptors across the HW queues,
            # while many small dma_starts serialize on the ~0.6us-per-dispatch
            # sequencer. The two engines stream in parallel.
            kbig = consts.tile([128, n_m, D], FP32)
            k_r = k_d.rearrange("(m p) d -> p m d", p=128)
            nc.sync.dma_start(out=kbig[:, 0:n_m // 2, :],
                              in_=k_r[:, 0:n_m // 2, :])
            nc.sync.dma_start(out=kbig[:, n_m // 2:n_m, :],
                              in_=k_r[:, n_m // 2:n_m, :])
            # query pair-major: qbig[:, c, b, :] is token tile (b*n_ch + c)
            qbig = consts.tile([128, n_ch, B, D], FP32)
            q_r = q_d.rearrange("(b c p) d -> p c b d", p=128, b=B)
            h_ch = max(1, n_ch // 2)
            for c0, c1 in ((0, h_ch), (h_ch, n_ch)):
                if c0 >= c1:
                    continue
                for bb in range(B):
                    nc.gpsimd.dma_start(out=qbig[:, c0:c1, bb, :],
                                        in_=q_r[:, c0:c1, bb, :])

            # ---------- key path (complete before the main loop) ----------
            for m in range(n_m):
                project_tile(kbig[:, m, :], m, wk_sb, bk_sb, KhT, on_act=True)
            psk2 = ps.tile([128, T], FP32, tag="score", bufs=2, name="psk2")
            nc.tensor.matmul(psk2[0:64, :], lhsT=wkh_sb, rhs=KhT[:, 0:T],
                             start=True, stop=True)
            nc.tensor.matmul(psk2[64:128, :], lhsT=wkh_sb, rhs=KhT[:, T:2 * T],
                             start=True, stop=True)
            nc.scalar.activation(k2, psk2, Identity, bias=b2col)
            # K_h token-major (lhsT of the heads matmul), bf16
            for bb in range(B):
                for k in range(n_sp):
                    tp2 = ps.tile([128, 512], FP32, tag="tp", bufs=4, name="tp2")
                    nc.tensor.transpose(
                        tp2[:, 0:DEPTH],
                        KhT[:, bb * T + 128 * k: bb * T + 128 * (k + 1)],
                        id_f32[0:64, 0:64])
                    nc.vector.tensor_copy(khb[:, bb, k, :], tp2[:, 0:DEPTH])

            def emit_query_pair(c, on_act):
                for bb in range(B):
                    project_tile(qbig[:, c, bb, :], bb * n_ch + c,
                                 wq_sb, bq_sb, QhT, on_act)
                psqb = ps.tile([128, 128], FP32, tag="pj", bufs=2, name="psqb")
                nc.tensor.matmul(psqb[0:64, :], lhsT=wqh_sb,
                                 rhs=QhT[:, 128 * c:128 * (c + 1)],
                                 start=True, stop=True)
                nc.tensor.matmul(psqb[64:128, :], lhsT=wqh_sb,
                                 rhs=QhT[:, T + 128 * c:T + 128 * (c + 1)],
                                 start=True, stop=True)
                dst = qb2[:, 128 * c:128 * (c + 1)]
                if on_act:
                    nc.scalar.copy(dst, psqb)
                else:
                    nc.vector.tensor_copy(dst, psqb)

            emit_query_pair(0, on_act=True)

            # output-projection constants, emitted AFTER the key stream so
            # the 1MB Wo load queues behind it (needed only at ~100us)
            wo_sb = consts.tile([128, 4, UNITS], FP32)
            nc.sync.dma_start(out=wo_sb, in_=wo_d.rearrange("(k p) n -> p k n", p=128))
            wo_bf = consts.tile([128, 4, UNITS], BF16)
            nc.vector.tensor_copy(wo_bf, wo_sb)
            bo_bc = consts.tile([128, UNITS], FP32)
            bo_bcast_ap = bass.AP(tensor=bo_d.ap().tensor, offset=0,
                                  ap=[[0, 128], [1, UNITS]])
            nc.sync.dma_start(out=bo_bc, in_=bo_bcast_ap)

            # ---------- main loop with streamed query + deferred stages ----
            def make_softmax(g):
                def emit():
                    score_tile = score_tiles.pop(g)
                    probs = sm.tile([128, T], FP32, tag="probs", name="probs")
                    nc.scalar.activation(probs, score_tile, Exp)
                    sums = sm.tile([128, 1], FP32, tag="sums", name="sums")
                    nc.vector.reduce_sum(sums, probs, axis=mybir.AxisListType.X)
                    rsum = sm.tile([128, 1], FP32, tag="rsum", name="rsum")
                    nc.vector.reciprocal(rsum, sums)
                    attn = sm.tile([128, T], BF16, tag="attn", name="attn")
                    nc.vector.tensor_scalar_mul(attn, probs, rsum)
                    tpsb = ps.tile([128, T], BF16, tag="tp", bufs=4, name="tpsb")
                    for k in range(n_sp):
                        nc.tensor.transpose(tpsb[:, 128 * k:128 * (k + 1)],
                                            attn[:, 128 * k:128 * (k + 1)],
                                            id_bf16)
                    nc.vector.tensor_copy(
                        attnT[:, :, g, :],
                        tpsb.rearrange("p (k r) -> p k r", k=n_sp))
                return emit

            attnT_jb = attnT.rearrange("p k g (j b) -> p k g j b", b=B)

            def make_chunk(g):
                def emit():
                    t0c = 64 * g
                    for bb in range(B):
                        psh = ps.tile([DEPTH, 64], FP32, tag="pj", bufs=2,
                                      name="psh")
                        for k in range(n_sp):
                            nc.tensor.matmul(
                                psh, lhsT=khb[:, bb, k, :],
                                rhs=attnT_jb[:, k, g, :, bb],
                                start=(k == 0), stop=(k == n_sp - 1))
                        nc.vector.tensor_copy(headsT[:, bb, t0c:t0c + 64], psh)
                    hb = dram.tile([DEPTH, B, 64], BF16, tag="hb", bufs=4,
                                   name="hb")
                    ms = dram.tile([NCORES * DEPTH, B, 64], BF16,
                                   addr_space="Shared", tag="ms", bufs=4,
                                   name="ms")
                    nc.sync.dma_start(out=hb, in_=headsT[:, :, t0c:t0c + 64])
                    nc.gpsimd.collective_compute(
                        "AllGather", mybir.AluOpType.bypass,
                        replica_groups=[list(range(NCORES))],
                        ins=[hb.opt()], outs=[ms.opt()])
                    merged_c = io.tile([128, 4, B, 64], BF16, tag="merged_c",
                                       name="merged_c")
                    nc.sync.dma_start(
                        out=merged_c,
                        in_=ms.rearrange("(k p) b t -> p k b t", p=128))
                    for bb in range(B):
                        ops = ps.tile([DEPTH, UNITS], FP32, tag="pj", bufs=2,
                                      name="ops")
                        for kc in range(4):
                            nc.tensor.matmul(ops, lhsT=merged_c[:, kc, bb, :],
                                             rhs=wo_bf[:, kc, :],
                                             start=(kc == 0), stop=(kc == 3))
                        out_sb = outp.tile([DEPTH, UNITS], FP32, tag="out_sb",
                                           name="out_sb")
                        nc.vector.tensor_add(out_sb, ops, bo_bc[0:DEPTH, :])
                        nc.sync.dma_start(
                            out=out_d[bb * T + t0c:bb * T + t0c + 64, :],
                            in_=out_sb)
                return emit

            score_tiles = {}
            pending = []  # deferred softmax/chunk emitters
            for g in range(n_g):
                if g % 2 == 1 and g // 2 + 1 < n_ch:
                    # stream the NEXT chunk's query pair a full score tile
                    # ahead of its first use (drains on DVE, off ACT's FIFO)
                    emit_query_pair(g // 2 + 1, on_act=False)

                score_ps = ps.tile([128, T], FP32, tag="score", bufs=2,
                                   name="score_ps")
                score_tiles[g] = score_ps
                for grp in range(64 // GT):
                    sum_slab = slabs.tile([128, GT * T], BF16, tag="sum_slab",
                                          name="sum_slab")
                    for j in range(GT):
                        t = 64 * g + GT * grp + j
                        nc.vector.tensor_scalar_add(
                            sum_slab[:, j * T:(j + 1) * T], k2, qb2[:, t:t + 1])
                    tanh_slab = slabs.tile([128, GT * T], BF16, tag="tanh_slab",
                                           name="tanh_slab", bufs=3)
                    nc.scalar.activation(tanh_slab, sum_slab, Tanh)
                    for j in range(GT):
                        jj = GT * grp + j  # t index within this score tile
                        nc.tensor.matmul(
                            score_ps,
                            lhsT=va_g[:, 126 - 2 * jj:254 - 2 * jj],
                            rhs=tanh_slab[:, j * T:(j + 1) * T],
                            start=(jj == 0), stop=(jj == 63))
                    if grp == 0:
                        for fn in pending:
                            fn()
                        pending = []
                pending.append(make_softmax(g))
                pending.append(make_chunk(g))
            for fn in pending:
                fn()

    nc.compile()
    return nc


def make_in_maps(inputs, T=512):
    """Shard full inputs head-parallel: core h gets head h's parameters."""
    f32 = np.float32
    q = np.ascontiguousarray(np.asarray(inputs["query"], f32)[:, :T, :].reshape(B * T, D))
    k = np.ascontiguousarray(np.asarray(inputs["key"], f32)[:, :T, :].reshape(B * T, D))
    Wq = np.asarray(inputs["Wq"], f32)
    Wk = np.asarray(inputs["Wk"], f32)
    bq = np.asarray(inputs["bq"], f32)
    bk = np.asarray(inputs["bk"], f32)
    Wq_h = np.asarray(inputs["Wq_h"], f32)
    Wk_h = np.asarray(inputs["Wk_h"], f32)
    va_h = np.asarray(inputs["va_h"], f32)
    b_h = np.asarray(inputs["b_h"], f32)
    Wo = np.ascontiguousarray(np.asarray(inputs["Wo"], f32))
    bo = np.ascontiguousarray(np.asarray(inputs["bo"], f32).reshape(1, UNITS))

    in_maps = []
    for h in range(NCORES):
        sl = slice(h * DEPTH, (h + 1) * DEPTH)
        in_maps.append({
            "query": q,
            "key": k,
            "wq_s": np.ascontiguousarray(Wq[:, sl]),
            "wk_s": np.ascontiguousarray(Wk[:, sl]),
            "bq_s": np.ascontiguousarray(bq[sl].reshape(DEPTH, 1)),
            "bk_s": np.ascontiguousarray(bk[sl].reshape(DEPTH, 1)),
            "wq_h": np.ascontiguousarray(Wq_h[h]),
            "wk_h": np.ascontiguousarray(Wk_h[h]),
            "va": np.ascontiguousarray(va_h[h].reshape(DEPTH, 1)),
            "bh": np.ascontiguousarray(b_h[h].reshape(DEPTH, 1)),
            "wo": Wo,
            "bo": bo,
        })
    return in_maps


_NC_CACHE = {}


def kernel(**inputs) -> np.ndarray:
    T = 512
    if T not in _NC_CACHE:
        _NC_CACHE[T] = build_nc(T)
    nc = _NC_CACHE[T]
    in_maps = make_in_maps(inputs, T)
    res = run_bass_kernel_spmd(nc, in_maps, core_ids=list(range(NCORES)))
    out = np.asarray(res.results[0]["out"], np.float32)
    return out.reshape(B, T, UNITS)


if __name__ == "__main__":
    import reference
    inp = {k: np.asarray(v) for k, v in reference.setup_inputs().items()}
    expected = np.asarray(reference.reference(**inp))
    got = kernel(**inp)
    rel = np.linalg.norm(got - expected) / np.linalg.norm(expected)
    print("Relative error:", rel)
